# revision 7
# baseline (speedup 1.0000x reference)
"""Trainium2 Bass kernel v4 for nn_AttentionLayer (B=8, N=1024, D=1024, H=16).

Sharding: data-parallel over batch -- one batch element per NeuronCore (8 cores).

v4 changes vs v3 (353us):
  - x/source cast to bf16 on host: DRAM loads halve to 4MB/core and the
    LN chain runs at 2x DVE rate. (Everything downstream was already bf16.)
  - Phase A processes x and source as one interleaved 16-tile pipeline.
  - Weight slice loads on the SP HWDGE ring (idle during phase A); first
    pair's weights prefetched before phase A.
  - Pair loop restructured: scores emit hf-merged (the kT stationary tile
    is reused across both query halves -> fewer LDWEIGHTS stalls), then
    kq(p+1), v(p), attnv(p) -- only k(0)/q(0) gate the first scores.
"""

import numpy as np

import concourse.bass as bass
import concourse.tile as tile
from concourse import bacc, mybir
from concourse.bass_utils import run_bass_kernel_spmd

F32 = mybir.dt.float32
BF16 = mybir.dt.bfloat16
MM_DT = BF16

B, N, D, H = 8, 1024, 1024, 16
DH = D // H  # 64
EPS = 1e-5
THETA = 10000.0
NT = N // 128  # 8 token tiles
DT = D // 128  # 8 channel tiles
NPAIR = H // 2  # 8 head pairs == channel tiles
SCALE = float(DH) ** -0.5

_PAIRSWAP = []
for _i in range(16):
    _PAIRSWAP += [2 * _i + 1, 2 * _i]


def build_program(apply_final_affine):
    nc = bacc.Bacc("TRN2", target_bir_lowering=False, debug=False)

    x_d = nc.dram_tensor("x", [N, D], MM_DT, kind="ExternalInput").ap()
    s_d = nc.dram_tensor("src", [N, D], MM_DT, kind="ExternalInput").ap()
    # weights prepacked on host as [p, r, dk, c] = W[dk*128+r, p*128+c]
    wq_d = nc.dram_tensor("wq", [NPAIR, 128, DT, 128], MM_DT, kind="ExternalInput").ap()
    wk_d = nc.dram_tensor("wk", [NPAIR, 128, DT, 128], MM_DT, kind="ExternalInput").ap()
    wv_d = nc.dram_tensor("wv", [NPAIR, 128, DT, 128], MM_DT, kind="ExternalInput").ap()
    wo_d = nc.dram_tensor("wo", [D, D], MM_DT, kind="ExternalInput").ap()
    cosf_d = nc.dram_tensor("cosf", [128, N], F32, kind="ExternalInput").ap()
    sinf_d = nc.dram_tensor("sinf", [128, N], F32, kind="ExternalInput").ap()
    # bias tables [128, DT] (col t = d-tile t); *s = pair-swapped variant
    bqa_d = nc.dram_tensor("bqa", [128, DT], F32, kind="ExternalInput").ap()
    bqs_d = nc.dram_tensor("bqs", [128, DT], F32, kind="ExternalInput").ap()
    bka_d = nc.dram_tensor("bka", [128, DT], F32, kind="ExternalInput").ap()
    bks_d = nc.dram_tensor("bks", [128, DT], F32, kind="ExternalInput").ap()
    # row vectors for partition-broadcast loads: bv', ln_g, ln_b
    rows_d = nc.dram_tensor("rows", [3, D], F32, kind="ExternalInput").ap()

    out_d = nc.dram_tensor("out", [N, D], F32, kind="ExternalOutput").ap()
    # bounce rows: [idx] raw sums (1024 = 2 heads x 512), [16+idx] reciprocals
    rb_d = nc.dram_tensor("rbounce", [2 * NPAIR * 2, 1024], F32).ap()

    def bcast_row(r, p=128):
        return bass.AP(
            tensor=rows_d.tensor, offset=rows_d.offset + r * D, ap=[[0, p], [1, D]]
        )

    def wcol_slice(w_d, p):
        """DRAM AP: prepacked block p as [128, DT, 128]; 2KB/partition contig."""
        return bass.AP(
            tensor=w_d.tensor,
            offset=w_d.offset + p * 128 * D,
            ap=[[DT * 128, 128], [128, DT], [1, 128]],
        )

    with tile.TileContext(nc) as tc:
        # ---------- pools ----------
        const = tc.alloc_tile_pool(name="const", bufs=1, side="left")
        stp = tc.alloc_tile_pool(name="stp", bufs=4, side="left")
        mvp = tc.alloc_tile_pool(name="mvp", bufs=4, side="left")
        trig = tc.alloc_tile_pool(name="trig", bufs=1, side="left")
        qkv = tc.alloc_tile_pool(name="qkv", bufs=1, side="left")  # qT,kT,v_sb,aoT
        ntp = tc.alloc_tile_pool(name="ntp", bufs=1, side="left")  # snT,xnT
        psP = tc.alloc_tile_pool(name="psP", bufs=2, space="PSUM", side="left")
        psS = tc.alloc_tile_pool(name="psS", bufs=2, space="PSUM", side="left")
        psO = tc.alloc_tile_pool(name="psO", bufs=2, space="PSUM", side="left")

        ldp = tc.alloc_tile_pool(name="ldp", bufs=6, side="right")
        xsp = tc.alloc_tile_pool(name="xsp", bufs=2, side="right")
        wsp = tc.alloc_tile_pool(name="wsp", bufs=6, side="right")  # weight slices
        rope = tc.alloc_tile_pool(name="rope", bufs=2, side="right")
        eTp = tc.alloc_tile_pool(name="eTp", bufs=2, side="right")
        att = tc.alloc_tile_pool(name="att", bufs=2, side="right")
        bcp = tc.alloc_tile_pool(name="bcp", bufs=2, side="right")

        # ---- constants (tiny; SP queue before anything else needs it)
        eps_t = const.tile([128, 1], F32)
        nc.vector.memset(eps_t, EPS)
        ones128 = const.tile([128, 128], F32)
        nc.vector.memset(ones128, 1.0)
        bqa = const.tile([128, DT], F32)
        nc.sync.dma_start(bqa, bqa_d)
        bqs = const.tile([128, DT], F32)
        nc.sync.dma_start(bqs, bqs_d)
        bka = const.tile([128, DT], F32)
        nc.sync.dma_start(bka, bka_d)
        bks = const.tile([128, DT], F32)
        nc.sync.dma_start(bks, bks_d)
        bv_bc = const.tile([128, D], F32)
        nc.sync.dma_start(bv_bc, bcast_row(0))
        cosf = trig.tile([128, N], F32)
        nc.sync.dma_start(cosf, cosf_d)
        sinf = trig.tile([128, N], F32)
        nc.sync.dma_start(sinf, sinf_d)

        # ---- persistent attention operands
        qT = qkv.tile([128, DT, N], MM_DT, tag="qT")
        kT = qkv.tile([128, DT, N], MM_DT, tag="kT")
        v_sb = qkv.tile([128, NT, H, 65], MM_DT, tag="v_sb")
        aoT = qkv.tile([128, DT, N], MM_DT, tag="aoT")
        nc.vector.tensor_copy(
            v_sb[:, :, :, 64:65],
            ones128.rearrange("p (a b c) -> p a b c", a=NT, b=H, c=1),
        )

        def load_wslice(w_d, p):
            wt = wsp.tile([128, DT, 128], MM_DT, tag="ws")
            nc.sync.dma_start(wt, wcol_slice(w_d, p))
            return wt

        # prefetch pair-0 weights on the SP ring ahead of the transposes
        wk0_t = load_wslice(wk_d, 0)
        wq0_t = load_wslice(wq_d, 0)

        def ln_tile(xt, out_ap):
            """LayerNorm [128, D] over free dim -> out_ap (no gamma/beta)."""
            stats = stp.tile([128, 2, 6], F32, tag="stats")
            for g in range(2):
                nc.vector.bn_stats(stats[:, g, :], xt[:, g * 512:(g + 1) * 512])
            mv = mvp.tile([128, 2], F32, tag="mv")
            nc.vector.bn_aggr(mv, stats)
            nc.scalar.activation(
                mv[:, 1:2], mv[:, 1:2], mybir.ActivationFunctionType.Sqrt, bias=eps_t
            )
            nc.vector.reciprocal(mv[:, 1:2], mv[:, 1:2])
            nc.vector.tensor_scalar(
                out=out_ap,
                in0=xt,
                scalar1=mv[:, 0:1],
                scalar2=mv[:, 1:2],
                op0=mybir.AluOpType.subtract,
                op1=mybir.AluOpType.mult,
            )

        # ============ Phase A: LN + XBAR transpose (x+src interleaved) ======
        xnT = ntp.tile([128, DT, N], MM_DT, tag="xnT")
        snT = ntp.tile([128, DT, N], MM_DT, tag="snT")
        seq = [(x_d, xnT, t) for t in range(NT)] + [(s_d, snT, t) for t in range(NT)]
        PRE = 5
        lts = {}

        def issue_load(i):
            ap_d, _, t = seq[i]
            lt = ldp.tile([128, D], MM_DT, tag="ld")
            nc.scalar.dma_start(lt, ap_d[t * 128:(t + 1) * 128, :])
            lts[i] = lt

        for i in range(PRE):
            issue_load(i)
        for i in range(len(seq)):
            if i + PRE < len(seq):
                issue_load(i + PRE)
            _, dstT, t = seq[i]
            xt = lts.pop(i)
            xs = xsp.tile([128, D], MM_DT, tag="xs")
            ln_tile(xt, xs)
            nc.sync.dma_start(dstT[:, :, t * 128:(t + 1) * 128], xs, transpose=True)

        def qk_proj_pair(wt, srcT, dstT, ba, bs, p):
            """dstT[:, p, :] = RoPE(W[:, p-block].T @ srcT + bias)."""
            for hf in range(2):
                ns = slice(hf * 512, (hf + 1) * 512)
                ps = psP.tile([128, 512], F32, tag="psP")
                for dk in range(DT):
                    nc.tensor.matmul(
                        ps,
                        wt[:, dk, :],
                        srcT[:, dk, ns],
                        start=(dk == 0),
                        stop=(dk == DT - 1),
                    )
                qs = rope.tile([128, 512], F32, tag="qs")
                nc.vector.stream_shuffle(qs, ps, _PAIRSWAP)
                t1 = rope.tile([128, 512], F32, tag="t1")
                nc.vector.scalar_tensor_tensor(
                    out=t1,
                    in0=ps,
                    scalar=ba[:, p:p + 1],
                    in1=cosf[:, ns],
                    op0=mybir.AluOpType.add,
                    op1=mybir.AluOpType.mult,
                )
                t2 = rope.tile([128, 512], F32, tag="t2")
                nc.vector.scalar_tensor_tensor(
                    out=t2,
                    in0=qs,
                    scalar=bs[:, p:p + 1],
                    in1=sinf[:, ns],
                    op0=mybir.AluOpType.add,
                    op1=mybir.AluOpType.mult,
                )
                nc.gpsimd.tensor_add(dstT[:, p, ns], t1, t2)

        def kq_step(p, wk_t=None, wq_t=None):
            if wk_t is None:
                wk_t = load_wslice(wk_d, p)
            qk_proj_pair(wk_t, xnT, kT, bka, bks, p)
            if wq_t is None:
                wq_t = load_wslice(wq_d, p)
            qk_proj_pair(wq_t, snT, qT, bqa, bqs, p)

        def v_step(p):
            wt = load_wslice(wv_d, p)
            for tt in range(NT):
                ps = psP.tile([128, 512], F32, tag="psP", name="psV")
                for dk in range(DT):
                    nc.tensor.matmul(
                        ps[:, 0:128],
                        xnT[:, dk, tt * 128:(tt + 1) * 128],
                        wt[:, dk, :],
                        start=(dk == 0),
                        stop=(dk == DT - 1),
                    )
                nc.vector.tensor_add(
                    v_sb[:, tt, 2 * p:2 * p + 2, 0:64],
                    ps[:, 0:128].rearrange("q (j d) -> q j d", j=2),
                    bv_bc[:, p * 128:(p + 1) * 128].rearrange(
                        "q (j d) -> q j d", j=2
                    ),
                )

        def scores_step(p, eps0, eps1):
            """hf-merged scoresT + exp: kT stationary reused across halves."""
            for mb in range(NT):
                mbs = slice(mb * 128, (mb + 1) * 128)
                pa = psS.tile([128, 1024], F32, tag="psS")
                pb = psS.tile([128, 1024], F32, tag="psS")
                nc.tensor.matmul(
                    pa[:, 0:512], kT[0:64, p, mbs], qT[0:64, p, 0:512],
                    start=True, stop=True,
                )
                nc.tensor.matmul(
                    pb[:, 0:512], kT[0:64, p, mbs], qT[0:64, p, 512:1024],
                    start=True, stop=True,
                )
                nc.tensor.matmul(
                    pa[:, 512:1024], kT[64:128, p, mbs], qT[64:128, p, 0:512],
                    start=True, stop=True,
                )
                nc.tensor.matmul(
                    pb[:, 512:1024], kT[64:128, p, mbs], qT[64:128, p, 512:1024],
                    start=True, stop=True,
                )
                nc.scalar.activation(
                    eps0[mb // 2][:, mb % 2], pa,
                    mybir.ActivationFunctionType.Exp, scale=SCALE,
                )
                nc.scalar.activation(
                    eps1[mb // 2][:, mb % 2], pb,
                    mybir.ActivationFunctionType.Exp, scale=SCALE,
                )

        def attnv_step(p, hf, eps_):
            """attn @ v for pair p, half hf; normalize; write aoT."""
            he, ho = 2 * p, 2 * p + 1
            ns = slice(hf * 512, (hf + 1) * 512)
            idx = 2 * p + hf
            pso_e = psO.tile([128, 512], F32, tag="psO")
            pso_o = psO.tile([128, 512], F32, tag="psO")
            for mb in range(NT):
                e_mb = eps_[mb // 2][:, mb % 2]
                nc.tensor.matmul(
                    pso_e[0:65, :], v_sb[:, mb, he, :], e_mb[:, 0, :],
                    start=(mb == 0), stop=(mb == NT - 1),
                )
                nc.tensor.matmul(
                    pso_o[0:65, :], v_sb[:, mb, ho, :], e_mb[:, 1, :],
                    start=(mb == 0), stop=(mb == NT - 1),
                )
            # copy to SBUF (frees PSUM); batch the two heads' denominator rows
            r_sb = att.tile([128, 2, 512], F32, tag="r_sb")
            nc.vector.tensor_copy(r_sb[0:65, 0, :], pso_e[0:65, :])
            nc.vector.tensor_copy(r_sb[0:65, 1, :], pso_o[0:65, :])
            # bounce: [1024] sums -> [128, 8] for parallel reciprocal -> back
            nc.gpsimd.dma_start(rb_d[idx:idx + 1, :], r_sb[64:65, :, :])
            rt = att.tile([128, 8], F32, tag="rt")
            nc.gpsimd.dma_start(
                rt,
                bass.AP(
                    tensor=rb_d.tensor,
                    offset=rb_d.offset + idx * 1024,
                    ap=[[8, 128], [1, 8]],
                ),
            )
            nc.vector.reciprocal(rt, rt)
            nc.gpsimd.dma_start(
                bass.AP(
                    tensor=rb_d.tensor,
                    offset=rb_d.offset + (16 + idx) * 1024,
                    ap=[[8, 128], [1, 8]],
                ),
                rt,
            )
            bc = bcp.tile([64, 2, 512], F32, tag="bc")
            nc.gpsimd.dma_start(
                bc,
                bass.AP(
                    tensor=rb_d.tensor,
                    offset=rb_d.offset + (16 + idx) * 1024,
                    ap=[[0, 64], [1, 1024]],
                ),
            )
            nc.vector.tensor_mul(aoT[0:64, p, ns], r_sb[0:64, 0, :], bc[:, 0, :])
            tmp = att.tile([64, 512], MM_DT, tag="tmp")
            nc.vector.tensor_mul(tmp, r_sb[0:64, 1, :], bc[:, 1, :])
            nc.sync.dma_start(aoT[64:128, p, ns], tmp)

        # ============ Pipelined projections + attention ============
        kq_step(0, wk_t=wk0_t, wq_t=wq0_t)
        for p in range(NPAIR):
            eps0 = [
                eTp.tile([128, 2, 2, 512], MM_DT, tag=f"eT{i}", name=f"ep{i}")
                for i in range(4)
            ]
            eps1 = [
                eTp.tile([128, 2, 2, 512], MM_DT, tag=f"eT{i}", name=f"eq{i}")
                for i in range(4)
            ]
            scores_step(p, eps0, eps1)
            if p + 1 < NPAIR:
                kq_step(p + 1)
            v_step(p)
            attnv_step(p, 0, eps0)
            attnv_step(p, 1, eps1)

        for pl in (bcp, att, eTp, rope, wsp, xsp, ldp):
            pl.release()

        # ============ Phase D: out-proj + final LN ============
        wdp = tc.alloc_tile_pool(name="wdp", bufs=8, side="right")
        finp = tc.alloc_tile_pool(name="finp", bufs=2, side="right")
        gbp = tc.alloc_tile_pool(name="gbp", bufs=1, side="right")

        wo_t = []
        for dk in range(DT):
            wt = wdp.tile([128, D], MM_DT, tag="wo")
            nc.gpsimd.dma_start(wt, wo_d[dk * 128:(dk + 1) * 128, :])
            wo_t.append(wt)
        if apply_final_affine:
            g_bc = gbp.tile([128, D], F32, tag="g_bc")
            nc.sync.dma_start(g_bc, bcast_row(1))
            b_bc = gbp.tile([128, D], F32, tag="b_bc")
            nc.sync.dma_start(b_bc, bcast_row(2))

        for nt in range(NT):
            fin = finp.tile([128, D], F32, tag="fin")
            for hf in range(2):
                ds_ = slice(hf * 512, (hf + 1) * 512)
                ps = psS.tile([128, 1024], F32, tag="psS")
                for dk in range(DT):
                    nc.tensor.matmul(
                        ps[:, 0:512],
                        aoT[:, dk, nt * 128:(nt + 1) * 128],
                        wo_t[dk][:, ds_],
                        start=(dk == 0),
                        stop=(dk == DT - 1),
                    )
                nc.scalar.copy(fin[:, ds_], ps[:, 0:512])
            z = finp.tile([128, D], F32, tag="z")
            ln_tile(fin, z)
            if apply_final_affine:
                nc.vector.tensor_mul(z, z, g_bc)
                nc.vector.tensor_add(z, z, b_bc)
            nc.sync.dma_start(out_d[nt * 128:(nt + 1) * 128, :], z)

        for pl in (gbp, finp, wdp, psO, psS, psP, ntp, qkv, trig, mvp, stp, const):
            pl.release()

    nc.compile()
    return nc


_NC_CACHE = {}


def _get_nc(apply_final_affine=False):
    key = bool(apply_final_affine)
    if key not in _NC_CACHE:
        _NC_CACHE[key] = build_program(key)
    return _NC_CACHE[key]


def _host_prep(inputs):
    import ml_dtypes
    wire = ml_dtypes.bfloat16
    f64 = np.float64
    Wq = inputs["Wq"].astype(f64)
    Wk = inputs["Wk"].astype(f64)
    Wv = inputs["Wv"].astype(f64)

    wq = (inputs["nq_g"].astype(f64)[:, None] * Wq).astype(wire)
    wk = (inputs["nk_g"].astype(f64)[:, None] * Wk).astype(wire)
    wv = (inputs["nv_g"].astype(f64)[:, None] * Wv).astype(wire)
    bq = (inputs["nq_b"].astype(f64) @ Wq + inputs["bq"].astype(f64)).astype(np.float32)
    bk = (inputs["nk_b"].astype(f64) @ Wk + inputs["bk"].astype(f64)).astype(np.float32)
    bv = (inputs["nv_b"].astype(f64) @ Wv + inputs["bv"].astype(f64)).astype(np.float32)

    def pack_w(w):
        # [D, D] -> [p, r, dk, c] with w_prep[p, r, dk, c] = w[dk*128+r, p*128+c]
        return np.ascontiguousarray(
            w.reshape(DT, 128, NPAIR, 128).transpose(2, 1, 0, 3)
        )

    # rope tables
    freqs = (1.0 / THETA ** (np.arange(0, DH, 2, dtype=np.float32) / DH)).astype(
        np.float32
    )
    t = np.arange(N, dtype=np.float32)
    ang = np.outer(t, freqs).astype(np.float64)  # [N, 32]
    cos_t = np.cos(ang).astype(np.float32)
    sin_t = np.sin(ang).astype(np.float32)
    p = np.arange(128)
    i_of_p = (p % 64) // 2
    cosf = np.ascontiguousarray(cos_t[:, i_of_p].T)  # [128, N]
    sgn = np.where(p % 2 == 0, -1.0, 1.0).astype(np.float32)
    sinf = np.ascontiguousarray(sin_t[:, i_of_p].T * sgn[:, None]).astype(np.float32)

    def btab(b):
        tab = np.zeros((128, DT), np.float32)
        tabs = np.zeros((128, DT), np.float32)
        for td in range(DT):
            tab[:, td] = b[td * 128 + p]
            tabs[:, td] = b[td * 128 + (p ^ 1)]
        return tab, tabs

    bqa, bqs = btab(bq)
    bka, bks = btab(bk)

    rows = np.stack(
        [bv, inputs["ln_g"].astype(np.float32), inputs["ln_b"].astype(np.float32)]
    )
    wo_w = np.ascontiguousarray(inputs["Wo"].astype(np.float64)).astype(wire)

    apply_final_affine = not (
        np.all(inputs["ln_g"] == 1.0) and np.all(inputs["ln_b"] == 0.0)
    )
    return {
        "wq": pack_w(wq), "wk": pack_w(wk), "wv": pack_w(wv),
        "wo": wo_w,
        "cosf": cosf, "sinf": sinf,
        "bqa": bqa, "bqs": bqs, "bka": bka, "bks": bks,
        "rows": rows.astype(np.float32),
    }, apply_final_affine


def run(inputs, trace=False, tmpdir=None):
    import ml_dtypes
    shared, apply_final_affine = _host_prep(inputs)
    nc = _get_nc(apply_final_affine)
    wire = ml_dtypes.bfloat16
    x = np.asarray(inputs["x"], np.float32).astype(wire)
    src = np.asarray(inputs["source"], np.float32).astype(wire)
    in_maps = [
        {"x": np.ascontiguousarray(x[c]), "src": np.ascontiguousarray(src[c]), **shared}
        for c in range(B)
    ]
    res = run_bass_kernel_spmd(nc, in_maps, list(range(B)), trace=trace, tmpdir=tmpdir)
    out = np.stack([res.results[c]["out"] for c in range(B)]).astype(np.float32)
    return out, res


def kernel(**inputs):
    return run(inputs)[0]
